# revision 1
# baseline (speedup 1.0000x reference)
"""Trainium2 kernel for nn_CLIPVisionTower_GOPrune.

Splits the work as:
  - device (8 NeuronCores, SPMD): the memory-bound reduction of
    attn [8,16,577,577] (~170 MB). Core i reduces its 16 contiguous
    [577,577] attention matrices to per-column partial sums.
    Layout trick: 16*577*577 = 2308*2308, and 2308 = 4*577, so each
    core's flat slice is a [2308, 2308] f32 matrix whose every row is
    4 whole attention rows. The kernel accumulates 128-row tiles
    elementwise on VectorE, leaving a [128, 4*577] accumulator that the
    host folds (f64) into exact-enough column sums.
  - host: the tiny top-k / windowed-max pruning logic on the [576]
    score vector, plus the [288, 1024] feature gather.

Output matches reference(): (feats [1,288,1024] f32, valid [288] bool).
"""

import numpy as np

NEG = -1e30
G, Q, W = 24, 12, 6
K = (Q * Q) // 4
L, H, S = 8, 16, 577          # layers, heads, seq
D = 1024
NCORES = 8
ROWS = 2308                   # per-core dram rows (of 2308 f32 each)
FREE = 2308                   # 4 * 577
TILES = 18                    # full [128, FREE] tiles; remainder is [4, FREE]

_CACHE = {}


def _build():
    import concourse.bacc as bacc
    import concourse.tile as tile
    import concourse.mybir as mybir

    f32 = mybir.dt.float32
    nc = bacc.Bacc("TRN2", target_bir_lowering=False, debug=False,
                   num_devices=NCORES)
    x = nc.dram_tensor("x", [ROWS, FREE], f32, kind="ExternalInput")
    out = nc.dram_tensor("out", [128, FREE], f32, kind="ExternalOutput")
    with tile.TileContext(nc) as tc:
        with tc.tile_pool(name="acc", bufs=1) as accp, \
             tc.tile_pool(name="inp", bufs=6) as inp:
            acc = accp.tile([128, FREE], f32)
            nc.sync.dma_start(acc[:, :], x[0:128, :])
            for t in range(1, TILES):
                tl = inp.tile([128, FREE], f32, tag="tl")
                nc.sync.dma_start(tl[:, :], x[128 * t:128 * (t + 1), :])
                nc.vector.tensor_add(acc[:, :], acc[:, :], tl[:, :])
            tl4 = inp.tile([4, FREE], f32, tag="tl")
            nc.sync.dma_start(tl4[:, :], x[128 * TILES:ROWS, :])
            nc.vector.tensor_add(acc[0:4, :], acc[0:4, :], tl4[:, :])
            nc.sync.dma_start(out[:, :], acc[:, :])
    nc.compile()
    return nc


def _get_nc():
    nc = _CACHE.get("nc")
    if nc is None:
        nc = _build()
        _CACHE["nc"] = nc
    return nc


def _device_colsums(attn):
    """Column sums of all L*H attention matrices, via the 8 cores."""
    from concourse.bass_utils import run_bass_kernel_spmd

    nc = _get_nc()
    a = np.ascontiguousarray(attn, dtype=np.float32).reshape(L * H, S * S)
    per = (L * H) // NCORES
    in_maps = [{"x": a[per * i:per * (i + 1)].reshape(ROWS, FREE)}
               for i in range(NCORES)]
    res = run_bass_kernel_spmd(nc, in_maps, core_ids=list(range(NCORES)))
    colsum = np.zeros(S, np.float64)
    for i in range(NCORES):
        o = res.results[i]["out"].astype(np.float64)
        colsum += o.reshape(128, FREE // S, S).sum(axis=(0, 1))
    return colsum


def _prune(score, tokens):
    """Reference pruning logic on an (accurate) score vector [576]."""
    sc = score.reshape(2, Q, 2, Q).transpose(0, 2, 1, 3).reshape(4, Q * Q)
    qr = np.arange(4) // 2
    qc = np.arange(4) % 2
    order_desc = np.argsort(-sc, axis=1, kind="stable")   # top_k tie-break
    top_idx = order_desc[:, :K]
    lr, lc = top_idx // Q, top_idx % Q
    imp_pos = (qr[:, None] * Q + lr) * G + (qc[:, None] * Q + lc)
    imp_mask = np.zeros((4, Q * Q), bool)
    imp_mask[np.arange(4)[:, None], top_idx] = True
    grid = np.where(imp_mask, NEG, sc).reshape(4, Q, Q)
    win = grid.reshape(4, W, 2, W, 2).transpose(0, 1, 3, 2, 4).reshape(4, W * W, 4)
    mx = win.max(axis=-1)
    idx = win.argmax(axis=-1)
    valid = mx > NEG * 0.5
    wr, wc = np.arange(W * W) // W, np.arange(W * W) % W
    frow = qr[:, None] * Q + 2 * wr[None, :] + idx // 2
    fcol = qc[:, None] * Q + 2 * wc[None, :] + idx % 2
    fpos = frow * G + fcol
    all_pos = np.concatenate([imp_pos.reshape(-1), fpos.reshape(-1)])
    all_valid = np.concatenate([np.ones(4 * K, bool), valid.reshape(-1)])
    sort_key = np.where(all_valid, all_pos.astype(np.int64),
                        10000 + np.arange(all_pos.shape[0]))
    order = np.argsort(sort_key, kind="stable")
    pos_sorted = all_pos[order]
    valid_sorted = all_valid[order]
    feats = tokens[np.clip(pos_sorted, 0, G * G - 1)]
    feats = np.where(valid_sorted[:, None], feats, np.float32(0.0))
    return feats.astype(np.float32)[None], valid_sorted


def kernel(attn, hidden):
    attn = np.asarray(attn)
    hidden = np.asarray(hidden, dtype=np.float32)
    colsum = _device_colsums(attn)
    score = colsum[1:] / float(L * H * S)
    tokens = hidden[0, 1:, :]
    return _prune(score, tokens)


# revision 2
# speedup vs baseline: 1.0459x; 1.0459x over previous
"""Trainium2 kernel for nn_CLIPVisionTower_GOPrune.

Work split:
  - device (8 NeuronCores, SPMD): the memory-bound reduction of
    attn [8,16,577,577] (~170 MB -> ~21.3 MB/core). Core i reduces its
    16 contiguous [577,577] attention matrices toward per-column sums.
    Layout trick: 16*577*577 = 2308*2308 and 2308 = 4*577, so each
    core's flat slice is a [2308, 2308] f32 matrix; a [128, 2308] tile
    row holds 4 whole attention rows, so free position f contributes to
    attention column f % 577.
    Per tile the free axis is split between two engines so neither is
    the bottleneck under the ~60 us/core DMA stream:
      - cols [0:1154): VectorE accumulates into acc[128, 1154]
      - cols [1154:2308): TensorE fp32 matmul with a ones[128,1] weight
        reduces over partitions into an accumulating PSUM [1, 1154]
    (fp32 matmul is exact: products are x*1.0, accumulation is f32.
     fp32r would be faster but is a rounded format - rejected because
     top-k selection must match the reference bit-for-bit.)
  - host: f64 combine of the per-core partials into exact-enough column
    sums, then the tiny top-k / windowed-max pruning logic and the
    [288, 1024] feature gather.

Output matches reference(): (feats [1,288,1024] f32, valid [288] bool).
"""

import numpy as np

NEG = -1e30
G, Q, W = 24, 12, 6
K = (Q * Q) // 4
L, H, S = 8, 16, 577          # layers, heads, seq
D = 1024
NCORES = 8
FREE = 2308                   # 4 * 577
ROWS = 2308                   # per-core dram rows of FREE f32
NT = 18                       # full [128, FREE] tiles
REM = ROWS - NT * 128         # 4-partition remainder tile
CD = 1154                     # DVE columns; PE handles FREE-CD
CPE = FREE - CD

_CACHE = {}


def _build():
    import concourse.bacc as bacc
    import concourse.tile as tile
    import concourse.mybir as mybir

    f32 = mybir.dt.float32
    nc = bacc.Bacc("TRN2", target_bir_lowering=False, debug=False,
                   num_devices=NCORES)
    x = nc.dram_tensor("x", [ROWS, FREE], f32, kind="ExternalInput")
    y1 = nc.dram_tensor("out_acc", [128, CD], f32, kind="ExternalOutput")
    y2 = nc.dram_tensor("out_pe", [1, CPE], f32, kind="ExternalOutput")
    chunks = []
    off = CD
    while off < FREE:
        n = min(512, FREE - off)
        chunks.append((off, off + n))
        off += n
    with tile.TileContext(nc) as tc:
        with tc.tile_pool(name="accp", bufs=1) as accp, \
             tc.tile_pool(name="inp", bufs=6) as inp, \
             tc.tile_pool(name="one", bufs=1) as onep, \
             tc.tile_pool(name="ps", bufs=1, space="PSUM") as psp, \
             tc.tile_pool(name="res", bufs=1) as resp:
            ones = onep.tile([128, 1], f32, name="ones")
            nc.vector.memset(ones[:, :], 1.0)
            ps = psp.tile([1, CPE], f32, name="ps")
            acc = accp.tile([128, CD], f32, name="acc")
            ntot = NT + (1 if REM else 0)
            for t in range(ntot):
                p = 128 if t < NT else REM
                tl = inp.tile([128, FREE], f32, name="tl", tag="tl")
                nc.sync.dma_start(tl[0:p, :], x[128 * t:128 * t + p, :])
                if t == 0:
                    nc.vector.tensor_copy(acc[:, :], tl[:, 0:CD])
                else:
                    nc.vector.tensor_add(acc[0:p, :], acc[0:p, :], tl[0:p, 0:CD])
                for c0, c1 in chunks:
                    nc.tensor.matmul(ps[:, c0 - CD:c1 - CD], ones[0:p, :],
                                     tl[0:p, c0:c1],
                                     start=(t == 0), stop=(t == ntot - 1))
            res = resp.tile([1, CPE], f32, name="res")
            nc.scalar.copy(res[:, :], ps[:, :])
            nc.sync.dma_start(y2[:, :], res[:, :])
            nc.sync.dma_start(y1[:, :], acc[:, :])
    nc.compile()
    return nc


def _get_nc():
    nc = _CACHE.get("nc")
    if nc is None:
        nc = _build()
        _CACHE["nc"] = nc
    return nc


def _device_colsums(attn):
    """f64 column sums of all L*H attention matrices, via the 8 cores."""
    from concourse.bass_utils import run_bass_kernel_spmd

    nc = _get_nc()
    a = np.ascontiguousarray(attn, dtype=np.float32).reshape(L * H, S * S)
    per = (L * H) // NCORES
    in_maps = [{"x": a[per * i:per * (i + 1)].reshape(ROWS, FREE)}
               for i in range(NCORES)]
    res = run_bass_kernel_spmd(nc, in_maps, core_ids=list(range(NCORES)))
    colsum = np.zeros(S, np.float64)
    for i in range(NCORES):
        av = res.results[i]["out_acc"].astype(np.float64)   # [128, CD]
        pv = res.results[i]["out_pe"][0].astype(np.float64)  # [CPE]
        colsum += av.reshape(128, CD // S, S).sum(axis=(0, 1))
        colsum += pv.reshape(CPE // S, S).sum(axis=0)
    return colsum


def _prune(score, tokens):
    """Reference pruning logic on an (accurate) score vector [576]."""
    sc = score.reshape(2, Q, 2, Q).transpose(0, 2, 1, 3).reshape(4, Q * Q)
    qr = np.arange(4) // 2
    qc = np.arange(4) % 2
    order_desc = np.argsort(-sc, axis=1, kind="stable")   # top_k tie-break
    top_idx = order_desc[:, :K]
    lr, lc = top_idx // Q, top_idx % Q
    imp_pos = (qr[:, None] * Q + lr) * G + (qc[:, None] * Q + lc)
    imp_mask = np.zeros((4, Q * Q), bool)
    imp_mask[np.arange(4)[:, None], top_idx] = True
    grid = np.where(imp_mask, NEG, sc).reshape(4, Q, Q)
    win = grid.reshape(4, W, 2, W, 2).transpose(0, 1, 3, 2, 4).reshape(4, W * W, 4)
    mx = win.max(axis=-1)
    idx = win.argmax(axis=-1)
    valid = mx > NEG * 0.5
    wr, wc = np.arange(W * W) // W, np.arange(W * W) % W
    frow = qr[:, None] * Q + 2 * wr[None, :] + idx // 2
    fcol = qc[:, None] * Q + 2 * wc[None, :] + idx % 2
    fpos = frow * G + fcol
    all_pos = np.concatenate([imp_pos.reshape(-1), fpos.reshape(-1)])
    all_valid = np.concatenate([np.ones(4 * K, bool), valid.reshape(-1)])
    sort_key = np.where(all_valid, all_pos.astype(np.int64),
                        10000 + np.arange(all_pos.shape[0]))
    order = np.argsort(sort_key, kind="stable")
    pos_sorted = all_pos[order]
    valid_sorted = all_valid[order]
    feats = tokens[np.clip(pos_sorted, 0, G * G - 1)]
    feats = np.where(valid_sorted[:, None], feats, np.float32(0.0))
    return feats.astype(np.float32)[None], valid_sorted


def kernel(attn, hidden):
    attn = np.asarray(attn)
    hidden = np.asarray(hidden, dtype=np.float32)
    colsum = _device_colsums(attn)
    score = colsum[1:] / float(L * H * S)
    tokens = hidden[0, 1:, :]
    return _prune(score, tokens)


# revision 4
# speedup vs baseline: 1.0991x; 1.0509x over previous
"""Trainium2 kernel for nn_CLIPVisionTower_GOPrune.

Work split:
  - device (8 NeuronCores, SPMD): the memory-bound reduction of
    attn [8,16,577,577] (~170 MB -> ~21.3 MB/core). Core i reduces its
    16 contiguous [577,577] attention matrices toward per-column sums.
    Layout trick: 16*577*577 = 4616*1154 and 1154 = 2*577, so each
    core's flat slice is a [4616, 1154] f32 matrix; every row is 2
    whole attention rows, so free position f of any row-aligned tile
    contributes to attention column (offset + f) % 577 with the offset
    a multiple of 577.
    Per [128, 1154] tile the free axis is split between two engines so
    neither bottlenecks the ~60 us/core DMA stream (DVE tensor_tensor
    pays a pipe-drain between ops on HW; PE fp32 matmul runs 4
    cycles/row; either alone would exceed the stream):
      - cols [0:642):    VectorE accumulates into acc[128, 642]
      - cols [642:1154): TensorE fp32 matmul with a ones[128,1] weight
        (one N=512 matmul per tile) reduces over partitions into an
        accumulating PSUM [1, 512]
    (fp32 matmul is exact here: products are x*1.0, accumulation is
     f32. fp32r would be 4x faster on PE but is a rounded format -
     rejected because top-k selection must match the reference.)
  - host: f64 combine of the per-core partials into exact-enough column
    sums, then the tiny top-k / windowed-max pruning logic and the
    [288, 1024] feature gather.

Output matches reference(): (feats [1,288,1024] f32, valid [288] bool).
"""

import numpy as np

NEG = -1e30
G, Q, W = 24, 12, 6
K = (Q * Q) // 4
L, H, S = 8, 16, 577          # layers, heads, seq
D = 1024
NCORES = 8
FREE = 1154                   # 2 * 577
ROWS = 4616                   # per-core dram rows of FREE f32
NFULL = ROWS // 128           # 36 full [128, FREE] tiles
REM = ROWS - NFULL * 128      # 8-partition remainder tile
CD = 642                      # DVE columns; PE handles FREE-CD = 512
CPE = FREE - CD

_CACHE = {}


def _build():
    import concourse.bacc as bacc
    import concourse.tile as tile
    import concourse.mybir as mybir

    f32 = mybir.dt.float32
    nc = bacc.Bacc("TRN2", target_bir_lowering=False, debug=False,
                   num_devices=NCORES)
    x = nc.dram_tensor("x", [ROWS, FREE], f32, kind="ExternalInput")
    y1 = nc.dram_tensor("out_acc", [128, CD], f32, kind="ExternalOutput")
    y2 = nc.dram_tensor("out_pe", [1, CPE], f32, kind="ExternalOutput")
    chunks = []
    off = CD
    while off < FREE:
        n = min(512, FREE - off)
        chunks.append((off, off + n))
        off += n
    with tile.TileContext(nc) as tc:
        with tc.tile_pool(name="accp", bufs=1) as accp, \
             tc.tile_pool(name="inp", bufs=8) as inp, \
             tc.tile_pool(name="one", bufs=1) as onep, \
             tc.tile_pool(name="ps", bufs=1, space="PSUM") as psp, \
             tc.tile_pool(name="res", bufs=1) as resp:
            ones = onep.tile([128, 1], f32, name="ones")
            nc.vector.memset(ones[:, :], 1.0)
            ps = psp.tile([1, CPE], f32, name="ps")
            acc = accp.tile([128, CD], f32, name="acc")
            # a full tile first (seeds acc across all 128 partitions via
            # copy), the remainder tile second, so the kernel tail still
            # ends on a full tile
            order = list(range(NFULL + (1 if REM else 0)))
            if REM:
                order = [order[0], order[-1]] + order[1:-1]
            for i, t in enumerate(order):
                p = 128 if t < NFULL else REM
                tl = inp.tile([128, FREE], f32, name="tl", tag="tl")
                nc.sync.dma_start(tl[0:p, :], x[128 * t:128 * t + p, :])
                if i == 0:
                    nc.vector.tensor_copy(acc[:, :], tl[:, 0:CD])
                else:
                    nc.vector.tensor_add(acc[0:p, :], acc[0:p, :], tl[0:p, 0:CD])
                for c0, c1 in chunks:
                    nc.tensor.matmul(ps[:, c0 - CD:c1 - CD], ones[0:p, :],
                                     tl[0:p, c0:c1],
                                     start=(i == 0), stop=(i == len(order) - 1))
            res = resp.tile([1, CPE], f32, name="res")
            nc.scalar.copy(res[:, :], ps[:, :])
            nc.sync.dma_start(y2[:, :], res[:, :])
            nc.sync.dma_start(y1[:, :], acc[:, :])
    nc.compile()
    return nc


def _get_nc():
    nc = _CACHE.get("nc")
    if nc is None:
        nc = _build()
        _CACHE["nc"] = nc
    return nc


def _device_colsums(attn):
    """f64 column sums of all L*H attention matrices, via the 8 cores."""
    from concourse.bass_utils import run_bass_kernel_spmd

    nc = _get_nc()
    a = np.ascontiguousarray(attn, dtype=np.float32).reshape(L * H, S * S)
    per = (L * H) // NCORES
    in_maps = [{"x": a[per * i:per * (i + 1)].reshape(ROWS, FREE)}
               for i in range(NCORES)]
    res = run_bass_kernel_spmd(nc, in_maps, core_ids=list(range(NCORES)))
    colsum = np.zeros(S, np.float64)
    for i in range(NCORES):
        av = res.results[i]["out_acc"].astype(np.float64)    # [128, CD]
        pv = res.results[i]["out_pe"][0].astype(np.float64)  # [CPE]
        # acc free position f -> column f % S (row offsets are multiples of S)
        colsum[:] += av[:, 0:S].sum(axis=0)
        np.add.at(colsum, np.arange(S, CD) % S, av[:, S:CD].sum(axis=0))
        # psum position j -> column (CD + j) % S
        np.add.at(colsum, (CD + np.arange(CPE)) % S, pv)
    return colsum


def _prune(score, tokens):
    """Reference pruning logic on an (accurate) score vector [576]."""
    sc = score.reshape(2, Q, 2, Q).transpose(0, 2, 1, 3).reshape(4, Q * Q)
    qr = np.arange(4) // 2
    qc = np.arange(4) % 2
    order_desc = np.argsort(-sc, axis=1, kind="stable")   # top_k tie-break
    top_idx = order_desc[:, :K]
    lr, lc = top_idx // Q, top_idx % Q
    imp_pos = (qr[:, None] * Q + lr) * G + (qc[:, None] * Q + lc)
    imp_mask = np.zeros((4, Q * Q), bool)
    imp_mask[np.arange(4)[:, None], top_idx] = True
    grid = np.where(imp_mask, NEG, sc).reshape(4, Q, Q)
    win = grid.reshape(4, W, 2, W, 2).transpose(0, 1, 3, 2, 4).reshape(4, W * W, 4)
    mx = win.max(axis=-1)
    idx = win.argmax(axis=-1)
    valid = mx > NEG * 0.5
    wr, wc = np.arange(W * W) // W, np.arange(W * W) % W
    frow = qr[:, None] * Q + 2 * wr[None, :] + idx // 2
    fcol = qc[:, None] * Q + 2 * wc[None, :] + idx % 2
    fpos = frow * G + fcol
    all_pos = np.concatenate([imp_pos.reshape(-1), fpos.reshape(-1)])
    all_valid = np.concatenate([np.ones(4 * K, bool), valid.reshape(-1)])
    sort_key = np.where(all_valid, all_pos.astype(np.int64),
                        10000 + np.arange(all_pos.shape[0]))
    order = np.argsort(sort_key, kind="stable")
    pos_sorted = all_pos[order]
    valid_sorted = all_valid[order]
    feats = tokens[np.clip(pos_sorted, 0, G * G - 1)]
    feats = np.where(valid_sorted[:, None], feats, np.float32(0.0))
    return feats.astype(np.float32)[None], valid_sorted


def kernel(attn, hidden):
    attn = np.asarray(attn)
    hidden = np.asarray(hidden, dtype=np.float32)
    colsum = _device_colsums(attn)
    score = colsum[1:] / float(L * H * S)
    tokens = hidden[0, 1:, :]
    return _prune(score, tokens)


# revision 5
# speedup vs baseline: 1.1085x; 1.0085x over previous
"""Trainium2 kernel for nn_CLIPVisionTower_GOPrune.

Work split:
  - device (8 NeuronCores, SPMD): the memory-bound reduction of
    attn [8,16,577,577] (~170 MB -> ~21.3 MB/core). Core i reduces its
    16 contiguous [577,577] attention matrices toward per-column sums.
    Layout trick: 16*577*577 = 4616*1154 and 1154 = 2*577, so the
    core's flat slice is a [4616, 1154] f32 matrix; every row is 2
    whole attention rows, so free position f of any row-aligned tile
    contributes to attention column (row_offset + f) % 577 with
    row_offset always a multiple of 577.
    Each tile's free axis is split between two engines so neither
    bottlenecks the ~60 us/core DMA stream (DVE tensor_tensor pays a
    pipe-drain between ops on HW; PE fp32 matmul runs 4 cycles/row;
    either alone would exceed the stream):
      - 35 regular [128,1154] tiles: cols [0:642) VectorE-accumulated
        into acc[128,642]; cols [642:1154) reduced over partitions by
        one N=512 fp32 TensorE matmul (ones[128,1] weight) into an
        accumulating PSUM [1,512].
      - the last row-pair is processed as two [128,577] half-tiles
        with a smaller split (cols [0:289) DVE, [289:577) PE into a
        second PSUM [1,288]) so every op on the post-stream critical
        path is short.
    acc[:, 289:642] is final after the last regular tile, so its
    output DMA overlaps the trailing compute.
    (fp32 matmul is exact here: products are x*1.0, accumulation is
     f32. fp32r would be 4x faster on PE but is a rounded format -
     rejected because top-k selection must match the reference.)
  - host: f64 combine of the per-core partials into exact-enough column
    sums, then the tiny top-k / windowed-max pruning logic and the
    [288, 1024] feature gather.

Output matches reference(): (feats [1,288,1024] f32, valid [288] bool).
"""

import numpy as np

NEG = -1e30
G, Q, W = 24, 12, 6
K = (Q * Q) // 4
L, H, S = 8, 16, 577          # layers, heads, seq
D = 1024
NCORES = 8
FREE = 1154                   # 2 * 577
ROWS = 4616                   # per-core dram rows of FREE f32
NFULL = ROWS // 128           # 36 [128, FREE] tiles; the last one is
NREG = NFULL - 1              # processed as two [128, S] half-tiles
REM = ROWS - NFULL * 128      # 8-partition remainder tile
CD = 642                      # regular-tile DVE cols; PE gets FREE-CD = 512
CPE = FREE - CD
CDH = 289                     # half-tile DVE cols; PE gets S-CDH = 288
CPEH = S - CDH

_CACHE = {}


def _build():
    import concourse.bacc as bacc
    import concourse.tile as tile
    import concourse.mybir as mybir

    f32 = mybir.dt.float32
    nc = bacc.Bacc("TRN2", target_bir_lowering=False, debug=False,
                   num_devices=NCORES)
    x = nc.dram_tensor("x", [ROWS, FREE], f32, kind="ExternalInput")
    y1 = nc.dram_tensor("out_acc", [128, CD], f32, kind="ExternalOutput")
    y2 = nc.dram_tensor("out_pe", [1, CPE + CPEH], f32, kind="ExternalOutput")
    xh = x.rearrange("r (a b) -> (r a) b", a=2)   # [2*ROWS, S] view
    with tile.TileContext(nc) as tc:
        with tc.tile_pool(name="accp", bufs=1) as accp, \
             tc.tile_pool(name="inp", bufs=8) as inp, \
             tc.tile_pool(name="one", bufs=1) as onep, \
             tc.tile_pool(name="ps", bufs=1, space="PSUM") as psp, \
             tc.tile_pool(name="ps2", bufs=1, space="PSUM") as psp2, \
             tc.tile_pool(name="res", bufs=1) as resp:
            ones = onep.tile([128, 1], f32, name="ones")
            nc.vector.memset(ones[:, :], 1.0)
            ps = psp.tile([1, CPE], f32, name="ps")
            ps2 = psp2.tile([1, CPEH], f32, name="ps2")
            acc = accp.tile([128, CD], f32, name="acc")
            # a full tile first (seeds acc across all 128 partitions via
            # copy), the 8-row remainder tile second
            order = list(range(NREG + 1))
            order = [order[0], order[-1]] + order[1:-1]
            last_reg = len(order) - 1
            for i, t in enumerate(order):
                p = 128 if t < NREG else REM
                row0 = 128 * t if t < NREG else 128 * NFULL
                tl = inp.tile([128, FREE], f32, name="tl", tag="tl")
                nc.sync.dma_start(tl[0:p, :], x[row0:row0 + p, :])
                if i == 0:
                    nc.vector.tensor_copy(acc[:, :], tl[:, 0:CD])
                else:
                    nc.vector.tensor_add(acc[0:p, :], acc[0:p, :], tl[0:p, 0:CD])
                nc.tensor.matmul(ps[:, :], ones[0:p, :], tl[0:p, CD:FREE],
                                 start=(i == 0), stop=(i == last_reg))
            # acc[:, CDH:CD] is final now; ship it while the trailing
            # half-tiles compute
            nc.sync.dma_start(y1[:, CDH:CD], acc[:, CDH:CD])
            for j in range(2):
                hr0 = 2 * 128 * NREG + 128 * j
                th = inp.tile([128, S], f32, name="th", tag="tl")
                nc.sync.dma_start(th[:, :], xh[hr0:hr0 + 128, :])
                nc.vector.tensor_add(acc[:, 0:CDH], acc[:, 0:CDH], th[:, 0:CDH])
                nc.tensor.matmul(ps2[:, :], ones[:, :], th[:, CDH:S],
                                 start=(j == 0), stop=(j == 1))
            res = resp.tile([1, CPE + CPEH], f32, name="res")
            nc.scalar.copy(res[:, 0:CPE], ps[:, :])
            nc.scalar.copy(res[:, CPE:CPE + CPEH], ps2[:, :])
            nc.sync.dma_start(y2[:, :], res[:, :])
            nc.sync.dma_start(y1[:, 0:CDH], acc[:, 0:CDH])
    nc.compile()
    return nc


def _get_nc():
    nc = _CACHE.get("nc")
    if nc is None:
        nc = _build()
        _CACHE["nc"] = nc
    return nc


def _device_colsums(attn):
    """f64 column sums of all L*H attention matrices, via the 8 cores."""
    from concourse.bass_utils import run_bass_kernel_spmd

    nc = _get_nc()
    a = np.ascontiguousarray(attn, dtype=np.float32).reshape(L * H, S * S)
    per = (L * H) // NCORES
    in_maps = [{"x": a[per * i:per * (i + 1)].reshape(ROWS, FREE)}
               for i in range(NCORES)]
    res = run_bass_kernel_spmd(nc, in_maps, core_ids=list(range(NCORES)))
    colsum = np.zeros(S, np.float64)
    for i in range(NCORES):
        av = res.results[i]["out_acc"].astype(np.float64)    # [128, CD]
        pv = res.results[i]["out_pe"][0].astype(np.float64)  # [CPE+CPEH]
        # acc position f -> column f % S
        colsum[:] += av[:, 0:S].sum(axis=0)
        np.add.at(colsum, np.arange(S, CD) % S, av[:, S:CD].sum(axis=0))
        # main psum position j -> column (CD + j) % S
        np.add.at(colsum, (CD + np.arange(CPE)) % S, pv[0:CPE])
        # trailing psum position j -> column CDH + j
        np.add.at(colsum, CDH + np.arange(CPEH), pv[CPE:CPE + CPEH])
    return colsum


def _prune(score, tokens):
    """Reference pruning logic on an (accurate) score vector [576]."""
    sc = score.reshape(2, Q, 2, Q).transpose(0, 2, 1, 3).reshape(4, Q * Q)
    qr = np.arange(4) // 2
    qc = np.arange(4) % 2
    order_desc = np.argsort(-sc, axis=1, kind="stable")   # top_k tie-break
    top_idx = order_desc[:, :K]
    lr, lc = top_idx // Q, top_idx % Q
    imp_pos = (qr[:, None] * Q + lr) * G + (qc[:, None] * Q + lc)
    imp_mask = np.zeros((4, Q * Q), bool)
    imp_mask[np.arange(4)[:, None], top_idx] = True
    grid = np.where(imp_mask, NEG, sc).reshape(4, Q, Q)
    win = grid.reshape(4, W, 2, W, 2).transpose(0, 1, 3, 2, 4).reshape(4, W * W, 4)
    mx = win.max(axis=-1)
    idx = win.argmax(axis=-1)
    valid = mx > NEG * 0.5
    wr, wc = np.arange(W * W) // W, np.arange(W * W) % W
    frow = qr[:, None] * Q + 2 * wr[None, :] + idx // 2
    fcol = qc[:, None] * Q + 2 * wc[None, :] + idx % 2
    fpos = frow * G + fcol
    all_pos = np.concatenate([imp_pos.reshape(-1), fpos.reshape(-1)])
    all_valid = np.concatenate([np.ones(4 * K, bool), valid.reshape(-1)])
    sort_key = np.where(all_valid, all_pos.astype(np.int64),
                        10000 + np.arange(all_pos.shape[0]))
    order = np.argsort(sort_key, kind="stable")
    pos_sorted = all_pos[order]
    valid_sorted = all_valid[order]
    feats = tokens[np.clip(pos_sorted, 0, G * G - 1)]
    feats = np.where(valid_sorted[:, None], feats, np.float32(0.0))
    return feats.astype(np.float32)[None], valid_sorted


def kernel(attn, hidden):
    attn = np.asarray(attn)
    hidden = np.asarray(hidden, dtype=np.float32)
    colsum = _device_colsums(attn)
    score = colsum[1:] / float(L * H * S)
    tokens = hidden[0, 1:, :]
    return _prune(score, tokens)


# revision 7
# speedup vs baseline: 1.1115x; 1.0027x over previous
"""Trainium2 kernel for nn_CLIPVisionTower_GOPrune.

Work split:
  - device (8 NeuronCores, SPMD): the memory-bound reduction of
    attn [8,16,577,577] (~170 MB -> ~21.3 MB/core). Core i reduces its
    16 contiguous [577,577] attention matrices toward per-column sums.
    Layout trick: 16*577*577 = 4616*1154 and 1154 = 2*577, so the
    core's flat slice is a [4616, 1154] f32 matrix; every row is 2
    whole attention rows, so free position f of any row-aligned tile
    contributes to attention column (row_offset + f) % 577 with
    row_offset always a multiple of 577.
    Each tile's free axis is split between two engines so neither
    bottlenecks the ~60 us/core DMA stream (DVE tensor_tensor pays a
    pipe-drain between ops on HW; PE fp32 matmul runs 4 cycles/row;
    either alone would exceed the stream):
      - 35 regular [128,1154] tiles: cols [0:642) VectorE-accumulated
        into acc[128,642]; cols [642:1154) reduced over partitions by
        one N=512 fp32 TensorE matmul (ones[128,1] weight) into an
        accumulating PSUM [1,512].
      - the last row-pair is processed as two [128,577] half-tiles
        with a smaller split (cols [0:289) DVE, [289:577) PE into a
        second PSUM [1,288]) so every op on the post-stream critical
        path is short.
    acc[:, 289:642] is final after the last regular tile, so its
    output DMA overlaps the trailing compute.
    (fp32 matmul is exact here: products are x*1.0, accumulation is
     f32. fp32r would be 4x faster on PE but is a rounded format -
     rejected because top-k selection must match the reference.)
  - host: f64 combine of the per-core partials into exact-enough column
    sums, then the tiny top-k / windowed-max pruning logic and the
    [288, 1024] feature gather.

Output matches reference(): (feats [1,288,1024] f32, valid [288] bool).
"""

import numpy as np

NEG = -1e30
G, Q, W = 24, 12, 6
K = (Q * Q) // 4
L, H, S = 8, 16, 577          # layers, heads, seq
D = 1024
NCORES = 8
FREE = 1154                   # 2 * 577
ROWS = 4616                   # per-core dram rows of FREE f32
NFULL = ROWS // 128           # 36 [128, FREE] tiles; the last one is
NREG = NFULL - 1              # processed as two [128, S] half-tiles
REM = ROWS - NFULL * 128      # 8-partition remainder tile
CD = 642                      # regular-tile DVE cols; PE gets FREE-CD = 512
CPE = FREE - CD
CDH = 353                     # half-tile DVE cols; PE gets S-CDH = 224
CPEH = S - CDH

_CACHE = {}


def _build():
    import concourse.bacc as bacc
    import concourse.tile as tile
    import concourse.mybir as mybir

    f32 = mybir.dt.float32
    nc = bacc.Bacc("TRN2", target_bir_lowering=False, debug=False,
                   num_devices=NCORES)
    x = nc.dram_tensor("x", [ROWS, FREE], f32, kind="ExternalInput")
    y1 = nc.dram_tensor("out_acc", [128, CD], f32, kind="ExternalOutput")
    y2 = nc.dram_tensor("out_pe", [1, CPE + CPEH], f32, kind="ExternalOutput")
    xh = x.rearrange("r (a b) -> (r a) b", a=2)   # [2*ROWS, S] view
    with tile.TileContext(nc) as tc:
        with tc.tile_pool(name="accp", bufs=1) as accp, \
             tc.tile_pool(name="inp", bufs=8) as inp, \
             tc.tile_pool(name="one", bufs=1) as onep, \
             tc.tile_pool(name="ps", bufs=1, space="PSUM") as psp, \
             tc.tile_pool(name="ps2", bufs=1, space="PSUM") as psp2, \
             tc.tile_pool(name="res", bufs=1) as resp:
            ones = onep.tile([128, 1], f32, name="ones")
            nc.vector.memset(ones[:, :], 1.0)
            ps = psp.tile([1, CPE], f32, name="ps")
            ps2 = psp2.tile([1, CPEH], f32, name="ps2")
            acc = accp.tile([128, CD], f32, name="acc")
            # a full tile first (seeds acc across all 128 partitions via
            # copy), the 8-row remainder tile second
            order = list(range(NREG + 1))
            order = [order[0], order[-1]] + order[1:-1]
            last_reg = len(order) - 1
            for i, t in enumerate(order):
                p = 128 if t < NREG else REM
                row0 = 128 * t if t < NREG else 128 * NFULL
                tl = inp.tile([128, FREE], f32, name="tl", tag="tl")
                nc.sync.dma_start(tl[0:p, :], x[row0:row0 + p, :])
                if i == 0:
                    nc.vector.tensor_copy(acc[:, :], tl[:, 0:CD])
                else:
                    nc.vector.tensor_add(acc[0:p, :], acc[0:p, :], tl[0:p, 0:CD])
                nc.tensor.matmul(ps[:, :], ones[0:p, :], tl[0:p, CD:FREE],
                                 start=(i == 0), stop=(i == last_reg))
            # acc[:, CDH:CD] is final now; ship it while the trailing
            # half-tiles compute
            nc.sync.dma_start(y1[:, CDH:CD], acc[:, CDH:CD])
            for j in range(2):
                hr0 = 2 * 128 * NREG + 128 * j
                th = inp.tile([128, S], f32, name="th", tag="tl")
                if j == 1:
                    # final half-tile: DVE columns land first; the PE
                    # columns are the very last DMA, so the (short) add
                    # chain and the matmul chain pipeline through the
                    # post-stream DMA-completion latency independently
                    nc.sync.dma_start(th[:, 0:CDH], xh[hr0:hr0 + 128, 0:CDH])
                    nc.vector.tensor_add(acc[:, 0:CDH], acc[:, 0:CDH],
                                         th[:, 0:CDH])
                    nc.sync.dma_start(th[:, CDH:S], xh[hr0:hr0 + 128, CDH:S])
                    nc.tensor.matmul(ps2[:, :], ones[:, :], th[:, CDH:S],
                                     start=False, stop=True)
                else:
                    nc.sync.dma_start(th[:, :], xh[hr0:hr0 + 128, :])
                    nc.vector.tensor_add(acc[:, 0:CDH], acc[:, 0:CDH],
                                         th[:, 0:CDH])
                    nc.tensor.matmul(ps2[:, :], ones[:, :], th[:, CDH:S],
                                     start=True, stop=False)
            res = resp.tile([1, CPE + CPEH], f32, name="res")
            nc.scalar.copy(res[:, 0:CPE], ps[:, :])
            nc.scalar.copy(res[:, CPE:CPE + CPEH], ps2[:, :])
            nc.sync.dma_start(y2[:, :], res[:, :])
            nc.sync.dma_start(y1[:, 0:CDH], acc[:, 0:CDH])
    nc.compile()
    return nc


def _get_nc():
    nc = _CACHE.get("nc")
    if nc is None:
        nc = _build()
        _CACHE["nc"] = nc
    return nc


def _device_colsums(attn):
    """f64 column sums of all L*H attention matrices, via the 8 cores."""
    from concourse.bass_utils import run_bass_kernel_spmd

    nc = _get_nc()
    a = np.ascontiguousarray(attn, dtype=np.float32).reshape(L * H, S * S)
    per = (L * H) // NCORES
    in_maps = [{"x": a[per * i:per * (i + 1)].reshape(ROWS, FREE)}
               for i in range(NCORES)]
    res = run_bass_kernel_spmd(nc, in_maps, core_ids=list(range(NCORES)))
    colsum = np.zeros(S, np.float64)
    for i in range(NCORES):
        av = res.results[i]["out_acc"].astype(np.float64)    # [128, CD]
        pv = res.results[i]["out_pe"][0].astype(np.float64)  # [CPE+CPEH]
        # acc position f -> column f % S
        colsum[:] += av[:, 0:S].sum(axis=0)
        np.add.at(colsum, np.arange(S, CD) % S, av[:, S:CD].sum(axis=0))
        # main psum position j -> column (CD + j) % S
        np.add.at(colsum, (CD + np.arange(CPE)) % S, pv[0:CPE])
        # trailing psum position j -> column CDH + j
        np.add.at(colsum, CDH + np.arange(CPEH), pv[CPE:CPE + CPEH])
    return colsum


def _prune(score, tokens):
    """Reference pruning logic on an (accurate) score vector [576]."""
    sc = score.reshape(2, Q, 2, Q).transpose(0, 2, 1, 3).reshape(4, Q * Q)
    qr = np.arange(4) // 2
    qc = np.arange(4) % 2
    order_desc = np.argsort(-sc, axis=1, kind="stable")   # top_k tie-break
    top_idx = order_desc[:, :K]
    lr, lc = top_idx // Q, top_idx % Q
    imp_pos = (qr[:, None] * Q + lr) * G + (qc[:, None] * Q + lc)
    imp_mask = np.zeros((4, Q * Q), bool)
    imp_mask[np.arange(4)[:, None], top_idx] = True
    grid = np.where(imp_mask, NEG, sc).reshape(4, Q, Q)
    win = grid.reshape(4, W, 2, W, 2).transpose(0, 1, 3, 2, 4).reshape(4, W * W, 4)
    mx = win.max(axis=-1)
    idx = win.argmax(axis=-1)
    valid = mx > NEG * 0.5
    wr, wc = np.arange(W * W) // W, np.arange(W * W) % W
    frow = qr[:, None] * Q + 2 * wr[None, :] + idx // 2
    fcol = qc[:, None] * Q + 2 * wc[None, :] + idx % 2
    fpos = frow * G + fcol
    all_pos = np.concatenate([imp_pos.reshape(-1), fpos.reshape(-1)])
    all_valid = np.concatenate([np.ones(4 * K, bool), valid.reshape(-1)])
    sort_key = np.where(all_valid, all_pos.astype(np.int64),
                        10000 + np.arange(all_pos.shape[0]))
    order = np.argsort(sort_key, kind="stable")
    pos_sorted = all_pos[order]
    valid_sorted = all_valid[order]
    feats = tokens[np.clip(pos_sorted, 0, G * G - 1)]
    feats = np.where(valid_sorted[:, None], feats, np.float32(0.0))
    return feats.astype(np.float32)[None], valid_sorted


def kernel(attn, hidden):
    attn = np.asarray(attn)
    hidden = np.asarray(hidden, dtype=np.float32)
    colsum = _device_colsums(attn)
    score = colsum[1:] / float(L * H * S)
    tokens = hidden[0, 1:, :]
    return _prune(score, tokens)
